# revision 28
# baseline (speedup 1.0000x reference)
"""CrossAttentionFusion kernel for Trainium2 (8 NeuronCores, data-parallel over batch).

Reference computation (per batch element, S=2048, D=512, HID=256):
  Q = l @ Wq + bq ; K = a @ Wk + bk ; V = a @ Wv + bv
  P = softmax(Q K^T / sqrt(D)) ; O = P @ V
  fused_l = gl*O + (2-gl)*l          (gl = sigmoid(alpha_l))
  fused_a = (1+ga)*a                 (ga = sigmoid(alpha_a))
  w = sigmoid(relu(v @ W1 + b1) @ W2 + b2) ; fused_v = w*v
  out = concat([fused_l, fused_a, fused_v], -1)     # [S, 3D]

Kernel strategy (per core, one batch element):
  - all large matmuls (K/Q/V projections, QK^T, PV) run in fp8e4 with the
    DoubleRow perf mode (2 k-subtiles per instruction, 2x bf16 throughput);
    the MLP gate path (h = relu(v@W1), h@W2) stays bf16 for accuracy.
  - scores are bounded, so softmax skips the max pass: P = exp(s)/16 (the
    1/16 keeps P inside fp8e4 range and cancels in the rowsum division),
    O = (P@[V|1]) with the rowsum from a ones-column appended to V.
  - compute inputs arrive as bf16 SWDGE cast-DMA chunks (transposed on PE
    via bf16 identity matmuls, cast to fp8 in the PSUM->SBUF copy); the
    elementwise epilogues (fused_a, fused_v, the l residual) read separate
    fp32 loads so those outputs stay near-exact.
  - phase A streams `a` (transpose + K^T + V + fused_a); then 4 rounds each
    process one l/v chunk (Q^T, h, visual weight, fused_v) followed by one
    512-row attention block, so attention matmuls overlap the l/v loads.
  - (2-gl)*l is parked in SBUF during the rounds, so the attention epilogue
    is reciprocal + two scaled copies + an add, with per-chunk batched
    output DMAs.
"""

import math
from contextlib import ExitStack

import ml_dtypes
import numpy as np

import concourse.bass as bass
import concourse.tile as tile
from concourse import bacc, mybir
from concourse.bass_utils import run_bass_kernel_spmd

B, S, D = 8, 2048, 512
HID = D // 2
P = 128  # partitions
NS = S // P          # 16 s-tiles
NC = D // P          # 4 d-chunks
NH = HID // P        # 2 hid-chunks
QB = 512             # q-block / s-chunk size
NQB = S // QB        # 4 chunks
TPC = QB // P        # 4 s-tiles per chunk
SCALE = 1.0 / math.sqrt(D)
LN16 = math.log(16.0)
DV = D + 1           # V width incl. ones column
N1 = 256             # PV split sizes
N2 = DV - N1         # 257

F32 = mybir.dt.float32
BF16 = mybir.dt.bfloat16
F8 = mybir.dt.float8e4
DR = mybir.MatmulPerfMode.DoubleRow


def build_kernel(gl: float, ga: float, b2val: float):
    nc = bacc.Bacc("TRN2", target_bir_lowering=False, debug=False, num_devices=8)

    a_t = nc.dram_tensor("a_t", [NS, P, D], F32, kind="ExternalInput").ap()
    l_t = nc.dram_tensor("l_t", [NS, P, D], F32, kind="ExternalInput").ap()
    v_t = nc.dram_tensor("v_t", [NS, P, D], F32, kind="ExternalInput").ap()
    wq = nc.dram_tensor("wq", [NC, P, D], F32, kind="ExternalInput").ap()
    wk = nc.dram_tensor("wk", [NC, P, D], F32, kind="ExternalInput").ap()
    wv = nc.dram_tensor("wv", [NC, P, D], F32, kind="ExternalInput").ap()
    w1 = nc.dram_tensor("w1", [NC, P, HID], F32, kind="ExternalInput").ap()
    w2 = nc.dram_tensor("w2", [P, NH], F32, kind="ExternalInput").ap()
    bq = nc.dram_tensor("bq", [P, NC], F32, kind="ExternalInput").ap()
    bk = nc.dram_tensor("bk", [P, NC], F32, kind="ExternalInput").ap()
    bv = nc.dram_tensor("bv", [1, D], F32, kind="ExternalInput").ap()
    b1 = nc.dram_tensor("b1", [P, NH], F32, kind="ExternalInput").ap()
    ident_in = nc.dram_tensor("ident_in", [P, P], BF16, kind="ExternalInput").ap()
    out = nc.dram_tensor("out", [NS, P, 3 * D], F32, kind="ExternalOutput").ap()

    with tile.TileContext(nc) as tc:
        _emit(tc, a_t, l_t, v_t, wq, wk, wv, w1, w2, bq, bk, bv, b1, ident_in,
              out, gl, ga, b2val)

    nc.compile()
    return nc


def _emit(tc, a_t, l_t, v_t, wq, wk, wv, w1, w2, bq, bk, bv, b1, ident_in, out, gl, ga, b2val):
    nc = tc.nc
    AF = mybir.ActivationFunctionType
    OP = mybir.AluOpType

    ctx = ExitStack()
    consts = ctx.enter_context(tc.tile_pool(name="consts", bufs=1))
    persist = ctx.enter_context(tc.tile_pool(name="persist", bufs=1))
    stage = ctx.enter_context(tc.tile_pool(name="stage", bufs=2))
    cpool = ctx.enter_context(tc.tile_pool(name="chunk", bufs=2))
    ppool = ctx.enter_context(tc.tile_pool(name="ppool", bufs=1))
    psum_mm = ctx.enter_context(tc.tile_pool(name="psum_mm", bufs=3, space="PSUM"))
    psum_att = ctx.enter_context(tc.tile_pool(name="psum_att", bufs=2, space="PSUM"))

    # ---- constants ----
    ident = consts.tile([P, P], BF16, tag="ident")
    nc.sync.dma_start(out=ident[:], in_=ident_in)

    # HAM warm-up: dependency-free matmuls so the PE clock ramps while the
    # first DMAs are streaming in.
    warm_in = consts.tile([P, P], BF16, tag="warm_in")
    nc.vector.memset(warm_in[:], 0.5)
    wps = psum_mm.tile([P, NC * P], F32, tag="mm")
    for _ in range(160):
        nc.tensor.matmul(
            wps[:, 0:P], lhsT=warm_in[:], rhs=warm_in[:], start=True, stop=True
        )

    # chunk 0 of `a` takes the HWDGE-f32 + DVE-cast path: the sync ring is
    # live several us before the Q7 SWDGE queue, so phase A can start early.
    a0_f32 = cpool.tile([P, TPC, D], F32, tag="lfs")
    src0 = a_t[0:TPC].rearrange("t p d -> p t d")
    nc.sync.dma_start(out=a0_f32[:], in_=src0)
    a0_bf = cpool.tile([P, TPC, D], BF16, tag="a_bf", bufs=3)
    for st4 in range(TPC):
        nc.vector.tensor_copy(a0_bf[:, st4, :], a0_f32[:, st4, :])

    # SWDGE queue order matters: every entry reads f32 from DRAM, so chunks
    # the pipeline needs early go ahead of the bulk weight loads.
    a1_bf = cpool.tile([P, TPC, D], BF16, tag="a_bf", bufs=3)
    nc.gpsimd.dma_start(
        out=a1_bf[:], in_=a_t[TPC : 2 * TPC].rearrange("t p d -> p t d")
    )
    a2_bf = cpool.tile([P, TPC, D], BF16, tag="a_bf", bufs=3)
    nc.gpsimd.dma_start(
        out=a2_bf[:], in_=a_t[2 * TPC : 3 * TPC].rearrange("t p d -> p t d")
    )

    # biases ([P, n] layouts prepared host-side) — tiny, keep them early
    bq_sb = consts.tile([P, NC], F32, tag="bq_sb")
    bk_sb = consts.tile([P, NC], F32, tag="bk_sb")
    b1_sb = consts.tile([P, NH], F32, tag="b1_sb")
    nc.gpsimd.dma_start(out=bq_sb[:], in_=bq)
    nc.gpsimd.dma_start(out=bk_sb[:], in_=bk)
    nc.gpsimd.dma_start(out=b1_sb[:], in_=b1)
    bv_bc = consts.tile([P, D], F32, tag="bv_bc")
    bv_bcast_ap = bass.AP(tensor=bv.tensor, offset=bv.offset, ap=[[0, P], bv.ap[1]])
    nc.gpsimd.dma_start(out=bv_bc[:], in_=bv_bcast_ap)
    nln16 = consts.tile([P, 1], F32, tag="nln16")
    nc.vector.memset(nln16[:], -LN16)

    # weights: SWDGE casts fp32->bf16 in the DMA; wk/wv/wq then go bf16->fp8
    # on DVE.
    wk_bf = consts.tile([P, NC, D], BF16, tag="wk_bf")
    wv_bf = consts.tile([P, NC, D], BF16, tag="wv_bf")
    wq_bf = consts.tile([P, NC, D], BF16, tag="wq_bf")
    w1_bf = consts.tile([P, NC, HID], BF16, tag="w1_bf")
    w2_bf = consts.tile([P, NH], BF16, tag="w2_bf")
    for dram, sb in ((wk, wk_bf), (wv, wv_bf)):
        for c in range(NC):
            nc.gpsimd.dma_start(out=sb[:, c, :], in_=dram[c])
    wk_f8 = consts.tile([P, NC, D], F8, tag="wk_f8")
    wv_f8 = consts.tile([P, NC, D], F8, tag="wv_f8")
    wq_f8 = consts.tile([P, NC, D], F8, tag="wq_f8")
    nc.vector.tensor_copy(wk_f8[:], wk_bf[:])
    nc.vector.tensor_copy(wv_f8[:], wv_bf[:])

    # a chunk 3 (waits for chunk-0's buffer), then round-0 l/v, then Q/MLP
    # weights (not needed until phase B)
    a3_bf = cpool.tile([P, TPC, D], BF16, tag="a_bf", bufs=3)
    nc.gpsimd.dma_start(
        out=a3_bf[:], in_=a_t[3 * TPC : 4 * TPC].rearrange("t p d -> p t d")
    )
    l0_bf = cpool.tile([P, TPC, D], BF16, tag="l_bf")
    nc.gpsimd.dma_start(out=l0_bf[:], in_=l_t[0:TPC].rearrange("t p d -> p t d"))
    v0_bf = cpool.tile([P, TPC, D], BF16, tag="v_bf")
    nc.gpsimd.dma_start(out=v0_bf[:], in_=v_t[0:TPC].rearrange("t p d -> p t d"))
    l0_fs = cpool.tile([P, TPC, D], F32, tag="lfs")
    nc.sync.dma_start(out=l0_fs[:], in_=l_t[0:TPC].rearrange("t p d -> p t d"))
    for c in range(NC):
        nc.gpsimd.dma_start(out=wq_bf[:, c, :], in_=wq[c])
        nc.gpsimd.dma_start(out=w1_bf[:, c, :], in_=w1[c])
    nc.gpsimd.dma_start(out=w2_bf[:], in_=w2)
    nc.vector.tensor_copy(wq_f8[:], wq_bf[:])

    # ---- persistent activations ----
    kT = persist.tile([P, NC, S], F8, tag="kT")         # K^T [d, s] fp8
    qT = persist.tile([P, NC, S], F8, tag="qT")         # Q^T [d, s] fp8
    v_sb = persist.tile([P, NS, DV], F8, tag="v_sb")    # [V | 1] natural fp8
    l_sc = persist.tile([P, NS, D], F32, tag="l_sc")    # (2-gl)*l, resident
    w_sb = persist.tile([P, NS], F32, tag="w_sb")       # visual weight per s-tile
    nc.vector.memset(v_sb[:, :, D:DV], 1.0)             # ones column

    def load_chunk_bf(dram, sc, tag):
        """One SWDGE cast-DMA: fp32 DRAM chunk -> bf16 [P, TPC, D] tile."""
        t = cpool.tile([P, TPC, D], BF16, tag=tag)
        src_ap = dram[sc * TPC : (sc + 1) * TPC].rearrange("t p d -> p t d")
        nc.gpsimd.dma_start(out=t[:], in_=src_ap)
        return t

    def load_chunk_f32(dram, sc, tag):
        """One HWDGE DMA: fp32 DRAM chunk -> fp32 [P, TPC, D] tile."""
        t = cpool.tile([P, TPC, D], F32, tag=tag)
        src_ap = dram[sc * TPC : (sc + 1) * TPC].rearrange("t p d -> p t d")
        nc.sync.dma_start(out=t[:], in_=src_ap)
        return t

    def store_chunk(eng, src, sc, col):
        """One DMA: [P, TPC, D] tile -> out[sc*TPC:(sc+1)*TPC, :, col:col+D]."""
        dst_ap = out[sc * TPC : (sc + 1) * TPC, :, col : col + D].rearrange(
            "t p d -> p t d"
        )
        eng.dma_start(out=dst_ap, in_=src[:])

    def transpose_tile(src, dstT, st4, eng):
        """Write transpose of bf16 [P, D] tile into dstT[:, :, st4*P:(st4+1)*P]
        via PE identity matmuls; the copy casts to dstT's dtype."""
        ps = psum_mm.tile([P, NC * P], F32, tag="mm")
        for c in range(NC):
            nc.tensor.matmul(
                ps[:, c * P : (c + 1) * P],
                lhsT=src[:, c * P : (c + 1) * P],
                rhs=ident[:],
                start=True,
                stop=True,
            )
        dst = dstT[:, :, st4 * P : (st4 + 1) * P]
        if eng == "scalar":
            nc.scalar.copy(dst, ps[:])
        else:
            nc.vector.tensor_copy(dst, ps[:])

    # ---- phase A: stream `a` -> aT, K^T, V, fused_a ----
    a_chunks = [a0_bf, a1_bf, a2_bf, a3_bf]
    for sc in range(NQB):
        a_bf = a_chunks[sc]
        aT = cpool.tile([P, NC, QB], F8, tag="aT")
        for st4 in range(TPC):
            transpose_tile(
                a_bf[:, st4, :], aT, st4, eng=("scalar" if st4 % 2 else "vector")
            )
        # fused_a = (1+ga)*a, one batched mul + store per chunk (bf16 source)
        oas = cpool.tile([P, TPC, D], F32, tag="oas")
        nc.vector.tensor_scalar_mul(out=oas[:], in0=a_bf[:], scalar1=1.0 + ga)
        store_chunk(nc.scalar, oas, sc, D)
        # K^T chunk columns (DoubleRow over ci pairs), bias on scalar ACT
        for co in range(NC):
            ps = psum_mm.tile([P, QB], F32, tag="mm")
            for cp in range(NC // 2):
                nc.tensor.matmul(
                    ps[:],
                    lhsT=wk_f8[:, 2 * cp : 2 * cp + 2, co * P : (co + 1) * P],
                    rhs=aT[:, 2 * cp : 2 * cp + 2, :],
                    start=(cp == 0),
                    stop=(cp == NC // 2 - 1),
                    perf_mode=DR,
                )
            nc.scalar.activation(
                out=kT[:, co, sc * QB : (sc + 1) * QB],
                in_=ps[:],
                func=AF.Identity,
                bias=bk_sb[:, co : co + 1],
                scale=1.0,
            )
        # V chunk rows (natural [s, d]), bias add on vector (bias varies
        # along the free dim so it must be a tensor_tensor)
        for st4 in range(TPC):
            st = sc * TPC + st4
            ps = psum_mm.tile([P, D], F32, tag="mm")
            for cp in range(NC // 2):
                nc.tensor.matmul(
                    ps[:],
                    lhsT=aT[:, 2 * cp : 2 * cp + 2, st4 * P : (st4 + 1) * P],
                    rhs=wv_f8[:, 2 * cp : 2 * cp + 2, :],
                    start=(cp == 0),
                    stop=(cp == NC // 2 - 1),
                    perf_mode=DR,
                )
            nc.vector.tensor_add(v_sb[:, st, 0:D], ps[:], bv_bc[:])

    # ---- phase B: rounds of (l/v chunk -> Q^T, h, w, fused_v) + attention ----
    l_pref = {0: l0_bf}
    v_pref = {0: v0_bf}
    lfs_pref = {0: l0_fs}
    for rb in range(NQB):
        # prefetch next round's chunks one round ahead
        if rb + 1 < NQB:
            l_pref[rb + 1] = load_chunk_bf(l_t, rb + 1, "l_bf")
            v_pref[rb + 1] = load_chunk_bf(v_t, rb + 1, "v_bf")
            lfs_pref[rb + 1] = load_chunk_f32(l_t, rb + 1, "lfs")
        l_bf, v_bf, lfs = l_pref[rb], v_pref[rb], lfs_pref[rb]
        lT = cpool.tile([P, NC, QB], F8, tag="lT")
        vT = cpool.tile([P, NC, QB], BF16, tag="vT")
        hT = cpool.tile([P, NH, QB], BF16, tag="hT")
        for st4 in range(TPC):
            transpose_tile(
                l_bf[:, st4, :], lT, st4, eng=("scalar" if st4 % 2 else "vector")
            )
            transpose_tile(
                v_bf[:, st4, :], vT, st4, eng=("vector" if st4 % 2 else "scalar")
            )
        # park (2-gl)*l for the attention epilogue (batched)
        nc.vector.tensor_scalar_mul(
            out=l_sc[:, rb * TPC : (rb + 1) * TPC, :], in0=lfs[:], scalar1=2.0 - gl
        )
        # Q^T chunk columns (DoubleRow), bias on scalar ACT
        for co in range(NC):
            ps = psum_mm.tile([P, QB], F32, tag="mm")
            for cp in range(NC // 2):
                nc.tensor.matmul(
                    ps[:],
                    lhsT=wq_f8[:, 2 * cp : 2 * cp + 2, co * P : (co + 1) * P],
                    rhs=lT[:, 2 * cp : 2 * cp + 2, :],
                    start=(cp == 0),
                    stop=(cp == NC // 2 - 1),
                    perf_mode=DR,
                )
            nc.scalar.activation(
                out=qT[:, co, rb * QB : (rb + 1) * QB],
                in_=ps[:],
                func=AF.Identity,
                bias=bq_sb[:, co : co + 1],
                scale=1.0,
            )
        # hT chunk = relu(W1^T vT + b1) in bf16
        for ch in range(NH):
            ps = psum_mm.tile([P, QB], F32, tag="mm")
            for ci in range(NC):
                nc.tensor.matmul(
                    ps[:],
                    lhsT=w1_bf[:, ci, ch * P : (ch + 1) * P],
                    rhs=vT[:, ci, :],
                    start=(ci == 0),
                    stop=(ci == NC - 1),
                )
            nc.scalar.activation(
                out=hT[:, ch, :],
                in_=ps[:],
                func=AF.Relu,
                bias=b1_sb[:, ch : ch + 1],
                scale=1.0,
            )
        # w = sigmoid(hT.T W2 + b2) = 0.5 + 0.5*tanh(0.5*(z+b2)); fused_v = w*v
        ovs = cpool.tile([P, TPC, D], F32, tag="ovs")
        for st4 in range(TPC):
            st = rb * TPC + st4
            psw = psum_att.tile([P, 1], F32, tag="w2", bufs=1)
            for ch in range(NH):
                nc.tensor.matmul(
                    psw[:],
                    lhsT=hT[:, ch, st4 * P : (st4 + 1) * P],
                    rhs=w2_bf[:, ch : ch + 1],
                    start=(ch == 0),
                    stop=(ch == NH - 1),
                )
            wt = stage.tile([P, 1], F32, tag="wt", bufs=2)
            nc.scalar.activation(
                out=wt[:], in_=psw[:], func=AF.Tanh, bias=0.5 * b2val, scale=0.5
            )
            nc.vector.tensor_scalar(
                out=w_sb[:, st : st + 1],
                in0=wt[:],
                scalar1=0.5,
                scalar2=0.5,
                op0=OP.mult,
                op1=OP.add,
            )
            nc.vector.tensor_scalar_mul(
                out=ovs[:, st4, :], in0=v_bf[:, st4, :], scalar1=w_sb[:, st : st + 1]
            )
        store_chunk(nc.scalar, ovs, rb, 2 * D)
        # attention block qb=rb
        qb = rb
        pT = ppool.tile([P, NS, QB], F8, tag="pT")
        for kt in range(NS):
            ps = psum_mm.tile([P, QB], F32, tag="mm")
            for cp in range(NC // 2):
                nc.tensor.matmul(
                    ps[:],
                    lhsT=kT[:, 2 * cp : 2 * cp + 2, kt * P : (kt + 1) * P],
                    rhs=qT[:, 2 * cp : 2 * cp + 2, qb * QB : (qb + 1) * QB],
                    start=(cp == 0),
                    stop=(cp == NC // 2 - 1),
                    perf_mode=DR,
                )
            nc.scalar.activation(
                out=pT[:, kt, :], in_=ps[:], func=AF.Exp, scale=SCALE, bias=nln16[:]
            )
        ols = cpool.tile([P, TPC, D], F32, tag="ols")
        for qt in range(TPC):
            qi = qb * TPC + qt
            pso1 = psum_att.tile([P, N1], F32, tag="o1")
            pso2 = psum_att.tile([P, N2], F32, tag="o2")
            for kp in range(NS // 2):
                nc.tensor.matmul(
                    pso1[:],
                    lhsT=pT[:, 2 * kp : 2 * kp + 2, qt * P : (qt + 1) * P],
                    rhs=v_sb[:, 2 * kp : 2 * kp + 2, 0:N1],
                    start=(kp == 0),
                    stop=(kp == NS // 2 - 1),
                    perf_mode=DR,
                )
                nc.tensor.matmul(
                    pso2[:],
                    lhsT=pT[:, 2 * kp : 2 * kp + 2, qt * P : (qt + 1) * P],
                    rhs=v_sb[:, 2 * kp : 2 * kp + 2, N1:DV],
                    start=(kp == 0),
                    stop=(kp == NS // 2 - 1),
                    perf_mode=DR,
                )
            rinv = stage.tile([P, 1], F32, tag="rinv", bufs=2)
            nc.vector.reciprocal(rinv[:], pso2[:, N2 - 1 : N2])
            t = stage.tile([P, D], F32, tag="t_l", bufs=2)
            nc.vector.tensor_scalar(
                out=t[:, 0:N1],
                in0=pso1[:],
                scalar1=rinv[:],
                scalar2=gl,
                op0=OP.mult,
                op1=OP.mult,
            )
            nc.vector.tensor_scalar(
                out=t[:, N1:D],
                in0=pso2[:, 0 : N2 - 1],
                scalar1=rinv[:],
                scalar2=gl,
                op0=OP.mult,
                op1=OP.mult,
            )
            nc.vector.tensor_add(ols[:, qt, :], t[:], l_sc[:, qi, :])
        store_chunk(nc.sync, ols, qb, 0)

    ctx.close()


def _execute(inputs, trace=False, **run_kwargs):
    a = np.ascontiguousarray(np.asarray(inputs["a"], dtype=np.float32))
    v = np.ascontiguousarray(np.asarray(inputs["v"], dtype=np.float32))
    l = np.ascontiguousarray(np.asarray(inputs["l"], dtype=np.float32))
    Wq = np.asarray(inputs["Wq"], dtype=np.float32)
    Wk = np.asarray(inputs["Wk"], dtype=np.float32)
    Wv = np.asarray(inputs["Wv"], dtype=np.float32)
    W1 = np.asarray(inputs["W1"], dtype=np.float32)
    W2 = np.asarray(inputs["W2"], dtype=np.float32)
    bq = np.asarray(inputs["bq"], dtype=np.float32)
    bk = np.asarray(inputs["bk"], dtype=np.float32)
    bv = np.asarray(inputs["bv"], dtype=np.float32)
    b1 = np.asarray(inputs["b1"], dtype=np.float32)
    b2 = np.asarray(inputs["b2"], dtype=np.float32)
    alpha_a = float(np.asarray(inputs["alpha_a"]))
    alpha_l = float(np.asarray(inputs["alpha_l"]))

    gl = float(1.0 / (1.0 + math.exp(-alpha_l)))
    ga = float(1.0 / (1.0 + math.exp(-alpha_a)))
    b2val = float(b2.reshape(-1)[0])

    nc = build_kernel(gl, ga, b2val)

    shared = {
        "wq": np.ascontiguousarray(Wq.reshape(NC, P, D)),
        "wk": np.ascontiguousarray(Wk.reshape(NC, P, D)),
        "wv": np.ascontiguousarray(Wv.reshape(NC, P, D)),
        "w1": np.ascontiguousarray(W1.reshape(NC, P, HID)),
        "w2": np.ascontiguousarray(W2.reshape(NH, P).T),
        "bq": np.ascontiguousarray(bq.reshape(NC, P).T),
        "bk": np.ascontiguousarray(bk.reshape(NC, P).T),
        "bv": np.ascontiguousarray(bv.reshape(1, D)),
        "b1": np.ascontiguousarray(b1.reshape(NH, P).T),
        "ident_in": np.eye(P, dtype=ml_dtypes.bfloat16),
    }
    in_maps = []
    for i in range(B):
        m = dict(shared)
        m["a_t"] = np.ascontiguousarray(a[i].reshape(NS, P, D))
        m["l_t"] = np.ascontiguousarray(l[i].reshape(NS, P, D))
        m["v_t"] = np.ascontiguousarray(v[i].reshape(NS, P, D))
        in_maps.append(m)

    res = run_bass_kernel_spmd(
        nc, in_maps, core_ids=list(range(B)), trace=trace, **run_kwargs
    )
    outs = [res.results[i]["out"].reshape(S, 3 * D) for i in range(B)]
    return np.stack(outs, axis=0).astype(np.float32), res


def kernel(**inputs) -> np.ndarray:
    out, _ = _execute(inputs, trace=False)
    return out


if __name__ == "__main__":
    print("kernel module OK")


# revision 31
# speedup vs baseline: 1.0733x; 1.0733x over previous
"""CrossAttentionFusion kernel for Trainium2 (8 NeuronCores, data-parallel over batch).

Reference computation (per batch element, S=2048, D=512, HID=256):
  Q = l @ Wq + bq ; K = a @ Wk + bk ; V = a @ Wv + bv
  P = softmax(Q K^T / sqrt(D)) ; O = P @ V
  fused_l = gl*O + (2-gl)*l          (gl = sigmoid(alpha_l))
  fused_a = (1+ga)*a                 (ga = sigmoid(alpha_a))
  w = sigmoid(relu(v @ W1 + b1) @ W2 + b2) ; fused_v = w*v
  out = concat([fused_l, fused_a, fused_v], -1)     # [S, 3D]

Kernel strategy (per core, one batch element):
  - all large matmuls (K/Q/V projections, QK^T, PV) run in fp8e4 with the
    DoubleRow perf mode (2 k-subtiles per instruction, 2x bf16 throughput);
    the MLP gate path (h = relu(v@W1), h@W2) stays bf16 for accuracy.
  - scores are bounded, so softmax skips the max pass: P = exp(s)/16 (the
    1/16 keeps P inside fp8e4 range and cancels in the rowsum division),
    O = (P@[V|1]) with the rowsum from a ones-column appended to V.
  - compute inputs arrive as bf16 SWDGE cast-DMA chunks (transposed on PE
    via bf16 identity matmuls, cast to fp8 in the PSUM->SBUF copy); the
    elementwise epilogues (fused_a, fused_v, the l residual) read separate
    fp32 loads so those outputs stay near-exact.
  - phase A streams `a` (transpose + K^T + V + fused_a); then 4 rounds each
    process one l/v chunk (Q^T, h, visual weight, fused_v) followed by one
    512-row attention block, so attention matmuls overlap the l/v loads.
  - (2-gl)*l is parked in SBUF during the rounds, so the attention epilogue
    is reciprocal + two scaled copies + an add, with per-chunk batched
    output DMAs.
"""

import math
from contextlib import ExitStack

import ml_dtypes
import numpy as np

import concourse.bass as bass
import concourse.tile as tile
from concourse import bacc, mybir
from concourse.bass_utils import run_bass_kernel_spmd

B, S, D = 8, 2048, 512
HID = D // 2
P = 128  # partitions
NS = S // P          # 16 s-tiles
NC = D // P          # 4 d-chunks
NH = HID // P        # 2 hid-chunks
QB = 512             # q-block / s-chunk size
NQB = S // QB        # 4 chunks
TPC = QB // P        # 4 s-tiles per chunk
SCALE = 1.0 / math.sqrt(D)
LN16 = math.log(16.0)
DV = D + 1           # V width incl. ones column
N1 = 256             # PV split sizes
N2 = DV - N1         # 257

F32 = mybir.dt.float32
BF16 = mybir.dt.bfloat16
F8 = mybir.dt.float8e4
DR = mybir.MatmulPerfMode.DoubleRow


def build_kernel(gl: float, ga: float, b2val: float):
    nc = bacc.Bacc("TRN2", target_bir_lowering=False, debug=False, num_devices=8)

    a_t = nc.dram_tensor("a_t", [NS, P, D], F32, kind="ExternalInput").ap()
    l_t = nc.dram_tensor("l_t", [NS, P, D], F32, kind="ExternalInput").ap()
    v_t = nc.dram_tensor("v_t", [NS, P, D], F32, kind="ExternalInput").ap()
    wq = nc.dram_tensor("wq", [NC, P, D], F32, kind="ExternalInput").ap()
    wk = nc.dram_tensor("wk", [NC, P, D], F32, kind="ExternalInput").ap()
    wv = nc.dram_tensor("wv", [NC, P, D], F32, kind="ExternalInput").ap()
    w1 = nc.dram_tensor("w1", [NC, P, HID], F32, kind="ExternalInput").ap()
    w2 = nc.dram_tensor("w2", [P, NH], F32, kind="ExternalInput").ap()
    bq = nc.dram_tensor("bq", [P, NC], F32, kind="ExternalInput").ap()
    bk = nc.dram_tensor("bk", [P, NC], F32, kind="ExternalInput").ap()
    bv = nc.dram_tensor("bv", [1, D], F32, kind="ExternalInput").ap()
    b1 = nc.dram_tensor("b1", [P, NH], F32, kind="ExternalInput").ap()
    ident_in = nc.dram_tensor("ident_in", [P, P], BF16, kind="ExternalInput").ap()
    out = nc.dram_tensor("out", [NS, P, 3 * D], F32, kind="ExternalOutput").ap()

    with tile.TileContext(nc) as tc:
        _emit(tc, a_t, l_t, v_t, wq, wk, wv, w1, w2, bq, bk, bv, b1, ident_in,
              out, gl, ga, b2val)

    nc.compile()
    return nc


def _emit(tc, a_t, l_t, v_t, wq, wk, wv, w1, w2, bq, bk, bv, b1, ident_in, out, gl, ga, b2val):
    nc = tc.nc
    AF = mybir.ActivationFunctionType
    OP = mybir.AluOpType

    ctx = ExitStack()
    consts = ctx.enter_context(tc.tile_pool(name="consts", bufs=1))
    persist = ctx.enter_context(tc.tile_pool(name="persist", bufs=1))
    stage = ctx.enter_context(tc.tile_pool(name="stage", bufs=2))
    cpool = ctx.enter_context(tc.tile_pool(name="chunk", bufs=2))
    ppool = ctx.enter_context(tc.tile_pool(name="ppool", bufs=1))
    psum_mm = ctx.enter_context(tc.tile_pool(name="psum_mm", bufs=3, space="PSUM"))
    psum_att = ctx.enter_context(tc.tile_pool(name="psum_att", bufs=2, space="PSUM"))

    # ---- constants ----
    ident = consts.tile([P, P], BF16, tag="ident")
    nc.sync.dma_start(out=ident[:], in_=ident_in)

    # HAM warm-up: dependency-free matmuls so the PE clock ramps while the
    # first DMAs are streaming in.
    warm_in = consts.tile([P, P], BF16, tag="warm_in")
    nc.vector.memset(warm_in[:], 0.5)
    wps = psum_mm.tile([P, NC * P], F32, tag="mm")
    for _ in range(160):
        nc.tensor.matmul(
            wps[:, 0:P], lhsT=warm_in[:], rhs=warm_in[:], start=True, stop=True
        )

    # chunk 0 of `a` takes the HWDGE-f32 + DVE-cast path: the sync ring is
    # live several us before the Q7 SWDGE queue, so phase A can start early.
    a0_f32 = cpool.tile([P, TPC, D], F32, tag="lfs")
    src0 = a_t[0:TPC].rearrange("t p d -> p t d")
    nc.sync.dma_start(out=a0_f32[:], in_=src0)
    a0_bf = cpool.tile([P, TPC, D], BF16, tag="a_bf", bufs=3)
    for st4 in range(TPC):
        nc.vector.tensor_copy(a0_bf[:, st4, :], a0_f32[:, st4, :])

    # biases ([P, n] layouts prepared host-side) — tiny, first on SWDGE
    bq_sb = consts.tile([P, NC], F32, tag="bq_sb")
    bk_sb = consts.tile([P, NC], F32, tag="bk_sb")
    b1_sb = consts.tile([P, NH], F32, tag="b1_sb")
    nc.gpsimd.dma_start(out=bq_sb[:], in_=bq)
    nc.gpsimd.dma_start(out=bk_sb[:], in_=bk)
    nc.gpsimd.dma_start(out=b1_sb[:], in_=b1)
    bv_bc = consts.tile([P, D], F32, tag="bv_bc")
    bv_bcast_ap = bass.AP(tensor=bv.tensor, offset=bv.offset, ap=[[0, P], bv.ap[1]])
    nc.gpsimd.dma_start(out=bv_bc[:], in_=bv_bcast_ap)
    nln16 = consts.tile([P, 1], F32, tag="nln16")
    nc.vector.memset(nln16[:], -LN16)

    # a chunks 1-3 on SWDGE (f32->bf16 cast in the DMA); chunk 3 reuses
    # chunk 0's buffer so its descriptor waits until chunk 0 is consumed
    a1_bf = cpool.tile([P, TPC, D], BF16, tag="a_bf", bufs=3)
    nc.gpsimd.dma_start(
        out=a1_bf[:], in_=a_t[TPC : 2 * TPC].rearrange("t p d -> p t d")
    )
    a2_bf = cpool.tile([P, TPC, D], BF16, tag="a_bf", bufs=3)
    nc.gpsimd.dma_start(
        out=a2_bf[:], in_=a_t[2 * TPC : 3 * TPC].rearrange("t p d -> p t d")
    )
    a3_bf = cpool.tile([P, TPC, D], BF16, tag="a_bf", bufs=3)
    nc.gpsimd.dma_start(
        out=a3_bf[:], in_=a_t[3 * TPC : 4 * TPC].rearrange("t p d -> p t d")
    )

    # wk/wv ride the early HWDGE queue as f32 (staged through the lfs-tag
    # rotation) so K/V projections of chunk 0 aren't gated on SWDGE startup
    wk_f32 = cpool.tile([P, NC, D], F32, tag="lfs")
    nc.sync.dma_start(out=wk_f32[:], in_=wk.rearrange("c p d -> p c d"))
    wv_f32 = cpool.tile([P, NC, D], F32, tag="lfs")
    nc.sync.dma_start(out=wv_f32[:], in_=wv.rearrange("c p d -> p c d"))
    wk_f8 = consts.tile([P, NC, D], F8, tag="wk_f8")
    wv_f8 = consts.tile([P, NC, D], F8, tag="wv_f8")
    wq_f8 = consts.tile([P, NC, D], F8, tag="wq_f8")
    nc.vector.tensor_copy(wk_f8[:], wk_f32[:])

    # Q/MLP weights via SWDGE bf16 (needed only from phase B), then round-0
    # l/v chunks
    wq_bf = consts.tile([P, NC, D], BF16, tag="wq_bf")
    w1_bf = consts.tile([P, NC, HID], BF16, tag="w1_bf")
    w2_bf = consts.tile([P, NH], BF16, tag="w2_bf")
    for c in range(NC):
        nc.gpsimd.dma_start(out=wq_bf[:, c, :], in_=wq[c])
        nc.gpsimd.dma_start(out=w1_bf[:, c, :], in_=w1[c])
    nc.gpsimd.dma_start(out=w2_bf[:], in_=w2)
    l0_bf = cpool.tile([P, TPC, D], BF16, tag="l_bf")
    nc.gpsimd.dma_start(out=l0_bf[:], in_=l_t[0:TPC].rearrange("t p d -> p t d"))
    v0_bf = cpool.tile([P, TPC, D], BF16, tag="v_bf")
    nc.gpsimd.dma_start(out=v0_bf[:], in_=v_t[0:TPC].rearrange("t p d -> p t d"))
    l0_fs = cpool.tile([P, TPC, D], F32, tag="lfs")
    nc.sync.dma_start(out=l0_fs[:], in_=l_t[0:TPC].rearrange("t p d -> p t d"))

    # ---- persistent activations ----
    kT = persist.tile([P, NC, S], F8, tag="kT")         # K^T [d, s] fp8
    qT = persist.tile([P, NC, S], F8, tag="qT")         # Q^T [d, s] fp8
    v_sb = persist.tile([P, NS, DV], F8, tag="v_sb")    # [V | 1] natural fp8
    l_sc = persist.tile([P, NS, D], F32, tag="l_sc")    # (2-gl)*l, resident
    w_sb = persist.tile([P, NS], F32, tag="w_sb")       # visual weight per s-tile
    nc.vector.memset(v_sb[:, :, D:DV], 1.0)             # ones column

    def load_chunk_bf(dram, sc, tag):
        """One SWDGE cast-DMA: fp32 DRAM chunk -> bf16 [P, TPC, D] tile."""
        t = cpool.tile([P, TPC, D], BF16, tag=tag)
        src_ap = dram[sc * TPC : (sc + 1) * TPC].rearrange("t p d -> p t d")
        nc.gpsimd.dma_start(out=t[:], in_=src_ap)
        return t

    def load_chunk_f32(dram, sc, tag):
        """One HWDGE DMA: fp32 DRAM chunk -> fp32 [P, TPC, D] tile."""
        t = cpool.tile([P, TPC, D], F32, tag=tag)
        src_ap = dram[sc * TPC : (sc + 1) * TPC].rearrange("t p d -> p t d")
        nc.sync.dma_start(out=t[:], in_=src_ap)
        return t

    def store_chunk(eng, src, sc, col):
        """One DMA: [P, TPC, D] tile -> out[sc*TPC:(sc+1)*TPC, :, col:col+D]."""
        dst_ap = out[sc * TPC : (sc + 1) * TPC, :, col : col + D].rearrange(
            "t p d -> p t d"
        )
        eng.dma_start(out=dst_ap, in_=src[:])

    def transpose_tile(src, dstT, st4, eng):
        """Write transpose of bf16 [P, D] tile into dstT[:, :, st4*P:(st4+1)*P]
        via PE identity matmuls; the copy casts to dstT's dtype."""
        ps = psum_mm.tile([P, NC * P], F32, tag="mm")
        for c in range(NC):
            nc.tensor.matmul(
                ps[:, c * P : (c + 1) * P],
                lhsT=src[:, c * P : (c + 1) * P],
                rhs=ident[:],
                start=True,
                stop=True,
            )
        dst = dstT[:, :, st4 * P : (st4 + 1) * P]
        if eng == "scalar":
            nc.scalar.copy(dst, ps[:])
        else:
            nc.vector.tensor_copy(dst, ps[:])

    # ---- phase A: stream `a` -> aT, K^T, V, fused_a ----
    a_chunks = [a0_bf, a1_bf, a2_bf, a3_bf]
    for sc in range(NQB):
        a_bf = a_chunks[sc]
        aT = cpool.tile([P, NC, QB], F8, tag="aT")
        for st4 in range(TPC):
            transpose_tile(
                a_bf[:, st4, :], aT, st4, eng=("scalar" if st4 % 2 else "vector")
            )
        if sc == 0:
            # placed here so the wv wait doesn't head-of-line block the
            # chunk-0 transpose copies on the vector queue
            nc.vector.tensor_copy(wv_f8[:], wv_f32[:])
        # fused_a = (1+ga)*a, one batched mul + store per chunk (bf16 source)
        oas = cpool.tile([P, TPC, D], F32, tag="oas")
        nc.vector.tensor_scalar_mul(out=oas[:], in0=a_bf[:], scalar1=1.0 + ga)
        store_chunk(nc.scalar, oas, sc, D)
        # K^T chunk columns (DoubleRow over ci pairs), bias on scalar ACT
        for co in range(NC):
            ps = psum_mm.tile([P, QB], F32, tag="mm")
            for cp in range(NC // 2):
                nc.tensor.matmul(
                    ps[:],
                    lhsT=wk_f8[:, 2 * cp : 2 * cp + 2, co * P : (co + 1) * P],
                    rhs=aT[:, 2 * cp : 2 * cp + 2, :],
                    start=(cp == 0),
                    stop=(cp == NC // 2 - 1),
                    perf_mode=DR,
                )
            nc.scalar.activation(
                out=kT[:, co, sc * QB : (sc + 1) * QB],
                in_=ps[:],
                func=AF.Identity,
                bias=bk_sb[:, co : co + 1],
                scale=1.0,
            )
        # V chunk rows (natural [s, d]), bias add on vector (bias varies
        # along the free dim so it must be a tensor_tensor)
        for st4 in range(TPC):
            st = sc * TPC + st4
            ps = psum_mm.tile([P, D], F32, tag="mm")
            for cp in range(NC // 2):
                nc.tensor.matmul(
                    ps[:],
                    lhsT=aT[:, 2 * cp : 2 * cp + 2, st4 * P : (st4 + 1) * P],
                    rhs=wv_f8[:, 2 * cp : 2 * cp + 2, :],
                    start=(cp == 0),
                    stop=(cp == NC // 2 - 1),
                    perf_mode=DR,
                )
            nc.vector.tensor_add(v_sb[:, st, 0:D], ps[:], bv_bc[:])

    # ---- phase B: rounds of (l/v chunk -> Q^T, h, w, fused_v) + attention ----
    l_pref = {0: l0_bf}
    v_pref = {0: v0_bf}
    lfs_pref = {0: l0_fs}
    for rb in range(NQB):
        # prefetch next round's chunks one round ahead
        if rb + 1 < NQB:
            l_pref[rb + 1] = load_chunk_bf(l_t, rb + 1, "l_bf")
            v_pref[rb + 1] = load_chunk_bf(v_t, rb + 1, "v_bf")
            lfs_pref[rb + 1] = load_chunk_f32(l_t, rb + 1, "lfs")
        l_bf, v_bf, lfs = l_pref[rb], v_pref[rb], lfs_pref[rb]
        if rb == 0:
            nc.vector.tensor_copy(wq_f8[:], wq_bf[:])
        lT = cpool.tile([P, NC, QB], F8, tag="lT")
        vT = cpool.tile([P, NC, QB], BF16, tag="vT")
        hT = cpool.tile([P, NH, QB], BF16, tag="hT")
        for st4 in range(TPC):
            transpose_tile(
                l_bf[:, st4, :], lT, st4, eng=("scalar" if st4 % 2 else "vector")
            )
            transpose_tile(
                v_bf[:, st4, :], vT, st4, eng=("vector" if st4 % 2 else "scalar")
            )
        # park (2-gl)*l for the attention epilogue (batched)
        nc.vector.tensor_scalar_mul(
            out=l_sc[:, rb * TPC : (rb + 1) * TPC, :], in0=lfs[:], scalar1=2.0 - gl
        )
        # Q^T chunk columns (DoubleRow), bias on scalar ACT
        for co in range(NC):
            ps = psum_mm.tile([P, QB], F32, tag="mm")
            for cp in range(NC // 2):
                nc.tensor.matmul(
                    ps[:],
                    lhsT=wq_f8[:, 2 * cp : 2 * cp + 2, co * P : (co + 1) * P],
                    rhs=lT[:, 2 * cp : 2 * cp + 2, :],
                    start=(cp == 0),
                    stop=(cp == NC // 2 - 1),
                    perf_mode=DR,
                )
            nc.scalar.activation(
                out=qT[:, co, rb * QB : (rb + 1) * QB],
                in_=ps[:],
                func=AF.Identity,
                bias=bq_sb[:, co : co + 1],
                scale=1.0,
            )
        # hT chunk = relu(W1^T vT + b1) in bf16
        for ch in range(NH):
            ps = psum_mm.tile([P, QB], F32, tag="mm")
            for ci in range(NC):
                nc.tensor.matmul(
                    ps[:],
                    lhsT=w1_bf[:, ci, ch * P : (ch + 1) * P],
                    rhs=vT[:, ci, :],
                    start=(ci == 0),
                    stop=(ci == NC - 1),
                )
            nc.scalar.activation(
                out=hT[:, ch, :],
                in_=ps[:],
                func=AF.Relu,
                bias=b1_sb[:, ch : ch + 1],
                scale=1.0,
            )
        # w = sigmoid(hT.T W2 + b2) = 0.5 + 0.5*tanh(0.5*(z+b2)); fused_v = w*v
        ovs = cpool.tile([P, TPC, D], F32, tag="ovs")
        for st4 in range(TPC):
            st = rb * TPC + st4
            psw = psum_att.tile([P, 1], F32, tag="w2", bufs=1)
            for ch in range(NH):
                nc.tensor.matmul(
                    psw[:],
                    lhsT=hT[:, ch, st4 * P : (st4 + 1) * P],
                    rhs=w2_bf[:, ch : ch + 1],
                    start=(ch == 0),
                    stop=(ch == NH - 1),
                )
            wt = stage.tile([P, 1], F32, tag="wt", bufs=2)
            nc.scalar.activation(
                out=wt[:], in_=psw[:], func=AF.Tanh, bias=0.5 * b2val, scale=0.5
            )
            nc.vector.tensor_scalar(
                out=w_sb[:, st : st + 1],
                in0=wt[:],
                scalar1=0.5,
                scalar2=0.5,
                op0=OP.mult,
                op1=OP.add,
            )
            nc.vector.tensor_scalar_mul(
                out=ovs[:, st4, :], in0=v_bf[:, st4, :], scalar1=w_sb[:, st : st + 1]
            )
        store_chunk(nc.scalar, ovs, rb, 2 * D)
        # attention block qb=rb
        qb = rb
        pT = ppool.tile([P, NS, QB], F8, tag="pT")
        for kt in range(NS):
            ps = psum_mm.tile([P, QB], F32, tag="mm")
            for cp in range(NC // 2):
                nc.tensor.matmul(
                    ps[:],
                    lhsT=kT[:, 2 * cp : 2 * cp + 2, kt * P : (kt + 1) * P],
                    rhs=qT[:, 2 * cp : 2 * cp + 2, qb * QB : (qb + 1) * QB],
                    start=(cp == 0),
                    stop=(cp == NC // 2 - 1),
                    perf_mode=DR,
                )
            nc.scalar.activation(
                out=pT[:, kt, :], in_=ps[:], func=AF.Exp, scale=SCALE, bias=nln16[:]
            )
        ols = cpool.tile([P, TPC, D], F32, tag="ols")
        for qt in range(TPC):
            qi = qb * TPC + qt
            pso1 = psum_att.tile([P, N1], F32, tag="o1")
            pso2 = psum_att.tile([P, N2], F32, tag="o2")
            for kp in range(NS // 2):
                nc.tensor.matmul(
                    pso1[:],
                    lhsT=pT[:, 2 * kp : 2 * kp + 2, qt * P : (qt + 1) * P],
                    rhs=v_sb[:, 2 * kp : 2 * kp + 2, 0:N1],
                    start=(kp == 0),
                    stop=(kp == NS // 2 - 1),
                    perf_mode=DR,
                )
                nc.tensor.matmul(
                    pso2[:],
                    lhsT=pT[:, 2 * kp : 2 * kp + 2, qt * P : (qt + 1) * P],
                    rhs=v_sb[:, 2 * kp : 2 * kp + 2, N1:DV],
                    start=(kp == 0),
                    stop=(kp == NS // 2 - 1),
                    perf_mode=DR,
                )
            rinv = stage.tile([P, 1], F32, tag="rinv", bufs=2)
            nc.vector.reciprocal(rinv[:], pso2[:, N2 - 1 : N2])
            t = stage.tile([P, D], F32, tag="t_l", bufs=2)
            nc.vector.tensor_scalar(
                out=t[:, 0:N1],
                in0=pso1[:],
                scalar1=rinv[:],
                scalar2=gl,
                op0=OP.mult,
                op1=OP.mult,
            )
            nc.vector.tensor_scalar(
                out=t[:, N1:D],
                in0=pso2[:, 0 : N2 - 1],
                scalar1=rinv[:],
                scalar2=gl,
                op0=OP.mult,
                op1=OP.mult,
            )
            nc.vector.tensor_add(ols[:, qt, :], t[:], l_sc[:, qi, :])
        store_chunk(nc.sync, ols, qb, 0)

    ctx.close()


def _execute(inputs, trace=False, **run_kwargs):
    a = np.ascontiguousarray(np.asarray(inputs["a"], dtype=np.float32))
    v = np.ascontiguousarray(np.asarray(inputs["v"], dtype=np.float32))
    l = np.ascontiguousarray(np.asarray(inputs["l"], dtype=np.float32))
    Wq = np.asarray(inputs["Wq"], dtype=np.float32)
    Wk = np.asarray(inputs["Wk"], dtype=np.float32)
    Wv = np.asarray(inputs["Wv"], dtype=np.float32)
    W1 = np.asarray(inputs["W1"], dtype=np.float32)
    W2 = np.asarray(inputs["W2"], dtype=np.float32)
    bq = np.asarray(inputs["bq"], dtype=np.float32)
    bk = np.asarray(inputs["bk"], dtype=np.float32)
    bv = np.asarray(inputs["bv"], dtype=np.float32)
    b1 = np.asarray(inputs["b1"], dtype=np.float32)
    b2 = np.asarray(inputs["b2"], dtype=np.float32)
    alpha_a = float(np.asarray(inputs["alpha_a"]))
    alpha_l = float(np.asarray(inputs["alpha_l"]))

    gl = float(1.0 / (1.0 + math.exp(-alpha_l)))
    ga = float(1.0 / (1.0 + math.exp(-alpha_a)))
    b2val = float(b2.reshape(-1)[0])

    nc = build_kernel(gl, ga, b2val)

    shared = {
        "wq": np.ascontiguousarray(Wq.reshape(NC, P, D)),
        "wk": np.ascontiguousarray(Wk.reshape(NC, P, D)),
        "wv": np.ascontiguousarray(Wv.reshape(NC, P, D)),
        "w1": np.ascontiguousarray(W1.reshape(NC, P, HID)),
        "w2": np.ascontiguousarray(W2.reshape(NH, P).T),
        "bq": np.ascontiguousarray(bq.reshape(NC, P).T),
        "bk": np.ascontiguousarray(bk.reshape(NC, P).T),
        "bv": np.ascontiguousarray(bv.reshape(1, D)),
        "b1": np.ascontiguousarray(b1.reshape(NH, P).T),
        "ident_in": np.eye(P, dtype=ml_dtypes.bfloat16),
    }
    in_maps = []
    for i in range(B):
        m = dict(shared)
        m["a_t"] = np.ascontiguousarray(a[i].reshape(NS, P, D))
        m["l_t"] = np.ascontiguousarray(l[i].reshape(NS, P, D))
        m["v_t"] = np.ascontiguousarray(v[i].reshape(NS, P, D))
        in_maps.append(m)

    res = run_bass_kernel_spmd(
        nc, in_maps, core_ids=list(range(B)), trace=trace, **run_kwargs
    )
    outs = [res.results[i]["out"].reshape(S, 3 * D) for i in range(B)]
    return np.stack(outs, axis=0).astype(np.float32), res


def kernel(**inputs) -> np.ndarray:
    out, _ = _execute(inputs, trace=False)
    return out


if __name__ == "__main__":
    print("kernel module OK")


# revision 37
# speedup vs baseline: 1.0792x; 1.0055x over previous
"""CrossAttentionFusion kernel for Trainium2 (8 NeuronCores, data-parallel over batch).

Reference computation (per batch element, S=2048, D=512, HID=256):
  Q = l @ Wq + bq ; K = a @ Wk + bk ; V = a @ Wv + bv
  P = softmax(Q K^T / sqrt(D)) ; O = P @ V
  fused_l = gl*O + (2-gl)*l          (gl = sigmoid(alpha_l))
  fused_a = (1+ga)*a                 (ga = sigmoid(alpha_a))
  w = sigmoid(relu(v @ W1 + b1) @ W2 + b2) ; fused_v = w*v
  out = concat([fused_l, fused_a, fused_v], -1)     # [S, 3D]

Kernel strategy (per core, one batch element):
  - all large matmuls (K/Q/V projections, QK^T, PV) run in fp8e4 with the
    DoubleRow perf mode (2 k-subtiles per instruction, 2x bf16 throughput);
    the MLP gate path (h = relu(v@W1), h@W2) stays bf16 for accuracy.
  - scores are bounded, so softmax skips the max pass: P = exp(s)/16 (the
    1/16 keeps P inside fp8e4 range and cancels in the rowsum division),
    O = (P@[V|1]) with the rowsum from a ones-column appended to V.
  - compute inputs arrive as bf16 SWDGE cast-DMA chunks (transposed on PE
    via bf16 identity matmuls, cast to fp8 in the PSUM->SBUF copy); the
    elementwise epilogues (fused_a, fused_v, the l residual) read separate
    fp32 loads so those outputs stay near-exact.
  - phase A streams `a` (transpose + K^T + V + fused_a); then 4 rounds each
    process one l/v chunk (Q^T, h, visual weight, fused_v) followed by one
    512-row attention block, so attention matmuls overlap the l/v loads.
  - (2-gl)*l is parked in SBUF during the rounds, so the attention epilogue
    is reciprocal + two scaled copies + an add, with per-chunk batched
    output DMAs.
"""

import math
from contextlib import ExitStack

import ml_dtypes
import numpy as np

import concourse.bass as bass
import concourse.tile as tile
from concourse import bacc, mybir
from concourse.bass_utils import run_bass_kernel_spmd

B, S, D = 8, 2048, 512
HID = D // 2
P = 128  # partitions
NS = S // P          # 16 s-tiles
NC = D // P          # 4 d-chunks
NH = HID // P        # 2 hid-chunks
QB = 512             # q-block / s-chunk size
NQB = S // QB        # 4 chunks
TPC = QB // P        # 4 s-tiles per chunk
SCALE = 1.0 / math.sqrt(D)
LN16 = math.log(16.0)
DV = D + 1           # V width incl. ones column
N1 = 256             # PV split sizes
N2 = DV - N1         # 257

F32 = mybir.dt.float32
BF16 = mybir.dt.bfloat16
F8 = mybir.dt.float8e4
DR = mybir.MatmulPerfMode.DoubleRow


def build_kernel(gl: float, ga: float, b2val: float):
    nc = bacc.Bacc("TRN2", target_bir_lowering=False, debug=False, num_devices=8)

    a_t = nc.dram_tensor("a_t", [NS, P, D], F32, kind="ExternalInput").ap()
    l_t = nc.dram_tensor("l_t", [NS, P, D], F32, kind="ExternalInput").ap()
    v_t = nc.dram_tensor("v_t", [NS, P, D], F32, kind="ExternalInput").ap()
    wq = nc.dram_tensor("wq", [NC, P, D], F32, kind="ExternalInput").ap()
    wk = nc.dram_tensor("wk", [NC, P, D], F32, kind="ExternalInput").ap()
    wv = nc.dram_tensor("wv", [NC, P, D], F32, kind="ExternalInput").ap()
    w1 = nc.dram_tensor("w1", [NC, P, HID], F32, kind="ExternalInput").ap()
    w2 = nc.dram_tensor("w2", [P, NH], F32, kind="ExternalInput").ap()
    bq = nc.dram_tensor("bq", [P, NC], F32, kind="ExternalInput").ap()
    bk = nc.dram_tensor("bk", [P, NC], F32, kind="ExternalInput").ap()
    bv = nc.dram_tensor("bv", [1, D], F32, kind="ExternalInput").ap()
    b1 = nc.dram_tensor("b1", [P, NH], F32, kind="ExternalInput").ap()
    ident_in = nc.dram_tensor("ident_in", [P, P], BF16, kind="ExternalInput").ap()
    out = nc.dram_tensor("out", [NS, P, 3 * D], F32, kind="ExternalOutput").ap()

    with tile.TileContext(nc) as tc:
        _emit(tc, a_t, l_t, v_t, wq, wk, wv, w1, w2, bq, bk, bv, b1, ident_in,
              out, gl, ga, b2val)

    nc.compile()
    return nc


def _emit(tc, a_t, l_t, v_t, wq, wk, wv, w1, w2, bq, bk, bv, b1, ident_in, out, gl, ga, b2val):
    nc = tc.nc
    AF = mybir.ActivationFunctionType
    OP = mybir.AluOpType

    ctx = ExitStack()
    consts = ctx.enter_context(tc.tile_pool(name="consts", bufs=1))
    persist = ctx.enter_context(tc.tile_pool(name="persist", bufs=1))
    stage = ctx.enter_context(tc.tile_pool(name="stage", bufs=2))
    cpool = ctx.enter_context(tc.tile_pool(name="chunk", bufs=2))
    ppool = ctx.enter_context(tc.tile_pool(name="ppool", bufs=1))
    psum_mm = ctx.enter_context(tc.tile_pool(name="psum_mm", bufs=3, space="PSUM"))
    psum_att = ctx.enter_context(tc.tile_pool(name="psum_att", bufs=2, space="PSUM"))

    # ---- constants ----
    ident = consts.tile([P, P], BF16, tag="ident")
    nc.sync.dma_start(out=ident[:], in_=ident_in)

    # HAM warm-up: dependency-free matmuls so the PE clock ramps while the
    # first DMAs are streaming in.
    warm_in = consts.tile([P, P], BF16, tag="warm_in")
    nc.vector.memset(warm_in[:], 0.5)
    wps = psum_mm.tile([P, NC * P], F32, tag="mm")
    for _ in range(160):
        nc.tensor.matmul(
            wps[:, 0:P], lhsT=warm_in[:], rhs=warm_in[:], start=True, stop=True
        )

    # all `a` chunks ride the early-live HWDGE queue as f32 and are
    # transposed in fp32 on the PE (4x transpose cycles, but no casts and
    # no SWDGE-startup dependency in phase A; fused_a stays exact)
    ident32 = consts.tile([P, P], F32, tag="ident32")
    nc.vector.tensor_copy(ident32[:], ident[:])

    def load_a(sc):
        t = cpool.tile([P, TPC, D], F32, tag="afs", bufs=3)
        nc.sync.dma_start(
            out=t[:], in_=a_t[sc * TPC : (sc + 1) * TPC].rearrange("t p d -> p t d")
        )
        return t

    a0_fs = load_a(0)

    # biases ([P, n] layouts prepared host-side) — tiny, first on SWDGE
    bq_sb = consts.tile([P, NC], F32, tag="bq_sb")
    bk_sb = consts.tile([P, NC], F32, tag="bk_sb")
    b1_sb = consts.tile([P, NH], F32, tag="b1_sb")
    nc.gpsimd.dma_start(out=bq_sb[:], in_=bq)
    nc.gpsimd.dma_start(out=bk_sb[:], in_=bk)
    nc.gpsimd.dma_start(out=b1_sb[:], in_=b1)
    bv_bc = consts.tile([P, D], F32, tag="bv_bc")
    bv_bcast_ap = bass.AP(tensor=bv.tensor, offset=bv.offset, ap=[[0, P], bv.ap[1]])
    nc.gpsimd.dma_start(out=bv_bc[:], in_=bv_bcast_ap)
    nln16 = consts.tile([P, 1], F32, tag="nln16")
    nc.vector.memset(nln16[:], -LN16)

    # wk/wv ride the early HWDGE queue as f32 (staged through the lfs-tag
    # rotation) so K/V projections of chunk 0 aren't gated on SWDGE startup
    wk_f32 = cpool.tile([P, NC, D], F32, tag="lfs")
    nc.sync.dma_start(out=wk_f32[:], in_=wk.rearrange("c p d -> p c d"))
    wv_f32 = cpool.tile([P, NC, D], F32, tag="lfs")
    nc.sync.dma_start(out=wv_f32[:], in_=wv.rearrange("c p d -> p c d"))
    a1_fs = load_a(1)
    a2_fs = load_a(2)
    a3_fs = load_a(3)
    wk_f8 = consts.tile([P, NC, D], F8, tag="wk_f8")
    wv_f8 = consts.tile([P, NC, D], F8, tag="wv_f8")
    wq_f8 = consts.tile([P, NC, D], F8, tag="wq_f8")
    nc.vector.tensor_copy(wk_f8[:], wk_f32[:])

    # Q/MLP weights via SWDGE bf16 (needed only from phase B), then round-0
    # l/v chunks
    wq_bf = consts.tile([P, NC, D], BF16, tag="wq_bf")
    w1_bf = consts.tile([P, NC, HID], BF16, tag="w1_bf")
    w2_bf = consts.tile([P, NH], BF16, tag="w2_bf")
    for c in range(NC):
        nc.gpsimd.dma_start(out=wq_bf[:, c, :], in_=wq[c])
        nc.gpsimd.dma_start(out=w1_bf[:, c, :], in_=w1[c])
    nc.gpsimd.dma_start(out=w2_bf[:], in_=w2)
    l0_bf = cpool.tile([P, TPC, D], BF16, tag="l_bf")
    nc.gpsimd.dma_start(out=l0_bf[:], in_=l_t[0:TPC].rearrange("t p d -> p t d"))
    v0_bf = cpool.tile([P, TPC, D], BF16, tag="v_bf")
    nc.gpsimd.dma_start(out=v0_bf[:], in_=v_t[0:TPC].rearrange("t p d -> p t d"))
    l0_fs = cpool.tile([P, TPC, D], F32, tag="lfs")
    nc.sync.dma_start(out=l0_fs[:], in_=l_t[0:TPC].rearrange("t p d -> p t d"))

    # ---- persistent activations ----
    kT = persist.tile([P, NC, S], F8, tag="kT")         # K^T [d, s] fp8
    qT = persist.tile([P, NC, S], F8, tag="qT")         # Q^T [d, s] fp8
    v_sb = persist.tile([P, NS, DV], F8, tag="v_sb")    # [V | 1] natural fp8
    l_sc = persist.tile([P, NS, D], F32, tag="l_sc")    # (2-gl)*l, resident
    w_sb = persist.tile([P, NS], F32, tag="w_sb")       # visual weight per s-tile
    nc.vector.memset(v_sb[:, :, D:DV], 1.0)             # ones column

    def load_chunk_bf(dram, sc, tag):
        """One SWDGE cast-DMA: fp32 DRAM chunk -> bf16 [P, TPC, D] tile."""
        t = cpool.tile([P, TPC, D], BF16, tag=tag)
        src_ap = dram[sc * TPC : (sc + 1) * TPC].rearrange("t p d -> p t d")
        nc.gpsimd.dma_start(out=t[:], in_=src_ap)
        return t

    def load_chunk_f32(dram, sc, tag):
        """One HWDGE DMA: fp32 DRAM chunk -> fp32 [P, TPC, D] tile."""
        t = cpool.tile([P, TPC, D], F32, tag=tag)
        src_ap = dram[sc * TPC : (sc + 1) * TPC].rearrange("t p d -> p t d")
        nc.sync.dma_start(out=t[:], in_=src_ap)
        return t

    def store_chunk(eng, src, sc, col):
        """One DMA: [P, TPC, D] tile -> out[sc*TPC:(sc+1)*TPC, :, col:col+D]."""
        dst_ap = out[sc * TPC : (sc + 1) * TPC, :, col : col + D].rearrange(
            "t p d -> p t d"
        )
        eng.dma_start(out=dst_ap, in_=src[:])

    def transpose_tile(src, dstT, st4, eng):
        """Write transpose of a [P, D] tile into dstT[:, :, st4*P:(st4+1)*P]
        via PE identity matmuls; the copy casts to dstT's dtype."""
        idn = ident32 if src.dtype == F32 else ident
        ps = psum_mm.tile([P, NC * P], F32, tag="mm")
        for c in range(NC):
            nc.tensor.matmul(
                ps[:, c * P : (c + 1) * P],
                lhsT=src[:, c * P : (c + 1) * P],
                rhs=idn[:],
                start=True,
                stop=True,
            )
        dst = dstT[:, :, st4 * P : (st4 + 1) * P]
        if eng == "scalar":
            nc.scalar.copy(dst, ps[:])
        else:
            nc.vector.tensor_copy(dst, ps[:])

    # ---- phase A: stream `a` -> aT, K^T, V, fused_a ----
    a_chunks = [a0_fs, a1_fs, a2_fs, a3_fs]
    for sc in range(NQB):
        a_fs = a_chunks[sc]
        aT = cpool.tile([P, NC, QB], F8, tag="aT")
        for st4 in range(TPC):
            transpose_tile(
                a_fs[:, st4, :], aT, st4, eng=("scalar" if st4 % 2 else "vector")
            )
        if sc == 0:
            # placed here so the wv wait doesn't head-of-line block the
            # chunk-0 transpose copies on the vector queue
            nc.vector.tensor_copy(wv_f8[:], wv_f32[:])
        # fused_a = (1+ga)*a, one batched mul + store per chunk (exact f32)
        oas = cpool.tile([P, TPC, D], F32, tag="oas", bufs=1)
        nc.vector.tensor_scalar_mul(out=oas[:], in0=a_fs[:], scalar1=1.0 + ga)
        store_chunk(nc.scalar, oas, sc, D)
        # K^T chunk columns (DoubleRow over ci pairs), bias on scalar ACT
        for co in range(NC):
            ps = psum_mm.tile([P, QB], F32, tag="mm")
            for cp in range(NC // 2):
                nc.tensor.matmul(
                    ps[:],
                    lhsT=wk_f8[:, 2 * cp : 2 * cp + 2, co * P : (co + 1) * P],
                    rhs=aT[:, 2 * cp : 2 * cp + 2, :],
                    start=(cp == 0),
                    stop=(cp == NC // 2 - 1),
                    perf_mode=DR,
                )
            nc.scalar.activation(
                out=kT[:, co, sc * QB : (sc + 1) * QB],
                in_=ps[:],
                func=AF.Identity,
                bias=bk_sb[:, co : co + 1],
                scale=1.0,
            )
        # V chunk rows (natural [s, d]), bias add on vector (bias varies
        # along the free dim so it must be a tensor_tensor)
        for st4 in range(TPC):
            st = sc * TPC + st4
            ps = psum_mm.tile([P, D], F32, tag="mm")
            for cp in range(NC // 2):
                nc.tensor.matmul(
                    ps[:],
                    lhsT=aT[:, 2 * cp : 2 * cp + 2, st4 * P : (st4 + 1) * P],
                    rhs=wv_f8[:, 2 * cp : 2 * cp + 2, :],
                    start=(cp == 0),
                    stop=(cp == NC // 2 - 1),
                    perf_mode=DR,
                )
            nc.vector.tensor_add(v_sb[:, st, 0:D], ps[:], bv_bc[:])

    # ---- phase B: rounds of (l/v chunk -> Q^T, h, w, fused_v) + attention ----
    l_pref = {0: l0_bf}
    v_pref = {0: v0_bf}
    lfs_pref = {0: l0_fs}
    for rb in range(NQB):
        # prefetch next round's chunks one round ahead
        if rb + 1 < NQB:
            l_pref[rb + 1] = load_chunk_bf(l_t, rb + 1, "l_bf")
            v_pref[rb + 1] = load_chunk_bf(v_t, rb + 1, "v_bf")
            lfs_pref[rb + 1] = load_chunk_f32(l_t, rb + 1, "lfs")
        l_bf, v_bf, lfs = l_pref[rb], v_pref[rb], lfs_pref[rb]
        if rb == 0:
            nc.vector.tensor_copy(wq_f8[:], wq_bf[:])
        lT = cpool.tile([P, NC, QB], F8, tag="lT")
        vT = cpool.tile([P, NC, QB], BF16, tag="vT")
        hT = cpool.tile([P, NH, QB], BF16, tag="hT")
        for st4 in range(TPC):
            transpose_tile(
                l_bf[:, st4, :], lT, st4, eng=("scalar" if st4 % 2 else "vector")
            )
            transpose_tile(
                v_bf[:, st4, :], vT, st4, eng=("vector" if st4 % 2 else "scalar")
            )
        # park (2-gl)*l for the attention epilogue (batched)
        nc.vector.tensor_scalar_mul(
            out=l_sc[:, rb * TPC : (rb + 1) * TPC, :], in0=lfs[:], scalar1=2.0 - gl
        )
        # Q^T chunk columns (DoubleRow), bias on scalar ACT
        for co in range(NC):
            ps = psum_mm.tile([P, QB], F32, tag="mm")
            for cp in range(NC // 2):
                nc.tensor.matmul(
                    ps[:],
                    lhsT=wq_f8[:, 2 * cp : 2 * cp + 2, co * P : (co + 1) * P],
                    rhs=lT[:, 2 * cp : 2 * cp + 2, :],
                    start=(cp == 0),
                    stop=(cp == NC // 2 - 1),
                    perf_mode=DR,
                )
            nc.scalar.activation(
                out=qT[:, co, rb * QB : (rb + 1) * QB],
                in_=ps[:],
                func=AF.Identity,
                bias=bq_sb[:, co : co + 1],
                scale=1.0,
            )
        # hT chunk = relu(W1^T vT + b1) in bf16
        for ch in range(NH):
            ps = psum_mm.tile([P, QB], F32, tag="mm")
            for ci in range(NC):
                nc.tensor.matmul(
                    ps[:],
                    lhsT=w1_bf[:, ci, ch * P : (ch + 1) * P],
                    rhs=vT[:, ci, :],
                    start=(ci == 0),
                    stop=(ci == NC - 1),
                )
            nc.scalar.activation(
                out=hT[:, ch, :],
                in_=ps[:],
                func=AF.Relu,
                bias=b1_sb[:, ch : ch + 1],
                scale=1.0,
            )
        # w = sigmoid(hT.T W2 + b2) = 0.5 + 0.5*tanh(0.5*(z+b2)); fused_v = w*v
        ovs = cpool.tile([P, TPC, D], F32, tag="ovs", bufs=1)
        for st4 in range(TPC):
            st = rb * TPC + st4
            psw = psum_att.tile([P, 1], F32, tag="w2", bufs=1)
            for ch in range(NH):
                nc.tensor.matmul(
                    psw[:],
                    lhsT=hT[:, ch, st4 * P : (st4 + 1) * P],
                    rhs=w2_bf[:, ch : ch + 1],
                    start=(ch == 0),
                    stop=(ch == NH - 1),
                )
            wt = stage.tile([P, 1], F32, tag="wt", bufs=2)
            nc.scalar.activation(
                out=wt[:], in_=psw[:], func=AF.Tanh, bias=0.5 * b2val, scale=0.5
            )
            nc.vector.tensor_scalar(
                out=w_sb[:, st : st + 1],
                in0=wt[:],
                scalar1=0.5,
                scalar2=0.5,
                op0=OP.mult,
                op1=OP.add,
            )
            nc.vector.tensor_scalar_mul(
                out=ovs[:, st4, :], in0=v_bf[:, st4, :], scalar1=w_sb[:, st : st + 1]
            )
        store_chunk(nc.scalar, ovs, rb, 2 * D)
        # attention block qb=rb
        qb = rb
        pT = ppool.tile([P, NS, QB], F8, tag="pT")
        for kt in range(NS):
            ps = psum_mm.tile([P, QB], F32, tag="mm")
            for cp in range(NC // 2):
                nc.tensor.matmul(
                    ps[:],
                    lhsT=kT[:, 2 * cp : 2 * cp + 2, kt * P : (kt + 1) * P],
                    rhs=qT[:, 2 * cp : 2 * cp + 2, qb * QB : (qb + 1) * QB],
                    start=(cp == 0),
                    stop=(cp == NC // 2 - 1),
                    perf_mode=DR,
                )
            nc.scalar.activation(
                out=pT[:, kt, :], in_=ps[:], func=AF.Exp, scale=SCALE, bias=nln16[:]
            )
        ols = cpool.tile([P, TPC, D], F32, tag="ols", bufs=1)
        for qt in range(TPC):
            qi = qb * TPC + qt
            pso1 = psum_att.tile([P, N1], F32, tag="o1")
            pso2 = psum_att.tile([P, N2], F32, tag="o2")
            for kp in range(NS // 2):
                nc.tensor.matmul(
                    pso1[:],
                    lhsT=pT[:, 2 * kp : 2 * kp + 2, qt * P : (qt + 1) * P],
                    rhs=v_sb[:, 2 * kp : 2 * kp + 2, 0:N1],
                    start=(kp == 0),
                    stop=(kp == NS // 2 - 1),
                    perf_mode=DR,
                )
                nc.tensor.matmul(
                    pso2[:],
                    lhsT=pT[:, 2 * kp : 2 * kp + 2, qt * P : (qt + 1) * P],
                    rhs=v_sb[:, 2 * kp : 2 * kp + 2, N1:DV],
                    start=(kp == 0),
                    stop=(kp == NS // 2 - 1),
                    perf_mode=DR,
                )
            rinv = stage.tile([P, 1], F32, tag="rinv", bufs=2)
            nc.vector.reciprocal(rinv[:], pso2[:, N2 - 1 : N2])
            t = stage.tile([P, D], F32, tag="t_l", bufs=2)
            nc.vector.tensor_scalar(
                out=t[:, 0:N1],
                in0=pso1[:],
                scalar1=rinv[:],
                scalar2=gl,
                op0=OP.mult,
                op1=OP.mult,
            )
            nc.vector.tensor_scalar(
                out=t[:, N1:D],
                in0=pso2[:, 0 : N2 - 1],
                scalar1=rinv[:],
                scalar2=gl,
                op0=OP.mult,
                op1=OP.mult,
            )
            nc.vector.tensor_add(ols[:, qt, :], t[:], l_sc[:, qi, :])
        store_chunk(nc.sync, ols, qb, 0)

    ctx.close()


def _execute(inputs, trace=False, **run_kwargs):
    a = np.ascontiguousarray(np.asarray(inputs["a"], dtype=np.float32))
    v = np.ascontiguousarray(np.asarray(inputs["v"], dtype=np.float32))
    l = np.ascontiguousarray(np.asarray(inputs["l"], dtype=np.float32))
    Wq = np.asarray(inputs["Wq"], dtype=np.float32)
    Wk = np.asarray(inputs["Wk"], dtype=np.float32)
    Wv = np.asarray(inputs["Wv"], dtype=np.float32)
    W1 = np.asarray(inputs["W1"], dtype=np.float32)
    W2 = np.asarray(inputs["W2"], dtype=np.float32)
    bq = np.asarray(inputs["bq"], dtype=np.float32)
    bk = np.asarray(inputs["bk"], dtype=np.float32)
    bv = np.asarray(inputs["bv"], dtype=np.float32)
    b1 = np.asarray(inputs["b1"], dtype=np.float32)
    b2 = np.asarray(inputs["b2"], dtype=np.float32)
    alpha_a = float(np.asarray(inputs["alpha_a"]))
    alpha_l = float(np.asarray(inputs["alpha_l"]))

    gl = float(1.0 / (1.0 + math.exp(-alpha_l)))
    ga = float(1.0 / (1.0 + math.exp(-alpha_a)))
    b2val = float(b2.reshape(-1)[0])

    nc = build_kernel(gl, ga, b2val)

    shared = {
        "wq": np.ascontiguousarray(Wq.reshape(NC, P, D)),
        "wk": np.ascontiguousarray(Wk.reshape(NC, P, D)),
        "wv": np.ascontiguousarray(Wv.reshape(NC, P, D)),
        "w1": np.ascontiguousarray(W1.reshape(NC, P, HID)),
        "w2": np.ascontiguousarray(W2.reshape(NH, P).T),
        "bq": np.ascontiguousarray(bq.reshape(NC, P).T),
        "bk": np.ascontiguousarray(bk.reshape(NC, P).T),
        "bv": np.ascontiguousarray(bv.reshape(1, D)),
        "b1": np.ascontiguousarray(b1.reshape(NH, P).T),
        "ident_in": np.eye(P, dtype=ml_dtypes.bfloat16),
    }
    in_maps = []
    for i in range(B):
        m = dict(shared)
        m["a_t"] = np.ascontiguousarray(a[i].reshape(NS, P, D))
        m["l_t"] = np.ascontiguousarray(l[i].reshape(NS, P, D))
        m["v_t"] = np.ascontiguousarray(v[i].reshape(NS, P, D))
        in_maps.append(m)

    res = run_bass_kernel_spmd(
        nc, in_maps, core_ids=list(range(B)), trace=trace, **run_kwargs
    )
    outs = [res.results[i]["out"].reshape(S, 3 * D) for i in range(B)]
    return np.stack(outs, axis=0).astype(np.float32), res


def kernel(**inputs) -> np.ndarray:
    out, _ = _execute(inputs, trace=False)
    return out


if __name__ == "__main__":
    print("kernel module OK")
